# revision 5
# baseline (speedup 1.0000x reference)
"""MoE routing kernel for trn2 (8 NeuronCores, expert-parallel).

Computes the dense-MoE reference:
    logits = x @ router_w; p = softmax(logits); top2 renormalized weights
    out = sum_e we[t,e] * (silu(x@w1[e]) * (x@v1[e])) @ w2[e]

Sharding: expert-parallel — core r holds expert r's weights, all tokens.
Each core computes its expert's weighted partial output out_e^T [D, T],
then a ReduceScatter over the 8 cores sums partials; core r keeps D-rows
[r*D/8, (r+1)*D/8). Host concatenates the shards and transposes.

Router is replicated on every core; the per-core expert weight column is
  we[t] = (l_e >= m2) * sigmoid(2*l_e - m1 - m2)
where m1/m2 are the top-2 logit values — exactly the renormalized top-2
softmax weight (full-softmax denominator cancels).

All matmuls run in float32r (fp32 data, 1 cycle/row on the PE vs 4 for
plain fp32; ~1.5e-4 matmul rel err measured on hw).
"""

import os

import numpy as np

import concourse.bass as bass
import concourse.mybir as mybir
import concourse.tile as tile
from concourse import bacc
from concourse.bass_utils import run_bass_kernel_spmd
from concourse.masks import make_identity

P = 128
N_CORES = 8
F32 = mybir.dt.float32
F32R = mybir.dt.float32r
AX = mybir.AxisListType
ALU = mybir.AluOpType
ACTF = mybir.ActivationFunctionType
BIG = 1.0e9


def _install_trace_hook_if_requested():
    """Optional: enables NTFF profiling when BASS_TRACE=1 (dev only)."""
    if os.environ.get("BASS_TRACE") != "1":
        return
    import sys
    import types

    if "antenv.axon_hooks" in sys.modules:
        return
    mod = types.ModuleType("antenv.axon_hooks")
    state = {"hook": None}
    mod.set_axon_ntff_profile_hook = lambda h: state.__setitem__("hook", h)
    mod.get_axon_ntff_profile_hook = lambda: state["hook"]
    sys.modules["antenv.axon_hooks"] = mod
    try:
        from trn_agent_boot.trn_boot import _ntff_profile_via_ctypes

        mod.set_axon_ntff_profile_hook(
            _ntff_profile_via_ctypes("/opt/axon/libaxon_pjrt.so")
        )
    except Exception:
        pass


def build(T, D, F, E, t_chunk):
    """Build the SPMD per-core bass program."""
    assert T % t_chunk == 0 and t_chunk % P == 0 and t_chunk <= 512
    assert D % P == 0 and F % P == 0
    DC = D // P          # contraction chunks over D
    FT = F // P          # f tiles (partition tiles of F)
    DT = D // P          # output d tiles
    TC = T // t_chunk    # token chunks
    NT = t_chunk // P    # token tiles per chunk
    DS = D // N_CORES    # output shard rows per core

    nc = bacc.Bacc("TRN2", target_bir_lowering=False, debug=False,
                   num_devices=N_CORES)

    xT = nc.dram_tensor("xT", [D, T], F32R, kind="ExternalInput")
    w1 = nc.dram_tensor("w1", [D, F], F32R, kind="ExternalInput")
    v1 = nc.dram_tensor("v1", [D, F], F32R, kind="ExternalInput")
    w2 = nc.dram_tensor("w2", [F, D], F32R, kind="ExternalInput")
    rw = nc.dram_tensor("rw", [D, E], F32R, kind="ExternalInput")
    eoh = nc.dram_tensor("eoh", [P, E], F32, kind="ExternalInput")
    out_shards = nc.dram_tensor("out_shards", [TC, DS, t_chunk], F32,
                                kind="ExternalOutput")

    with tile.TileContext(nc) as tc:
        with (
            tc.tile_pool(name="const", bufs=1) as const,
            tc.tile_pool(name="xpool", bufs=1) as xpool,
            tc.tile_pool(name="wpool", bufs=2) as wpool,
            tc.tile_pool(name="w2pool", bufs=2) as w2pool,
            tc.tile_pool(name="gpool", bufs=FT + 2) as gpool,
            tc.tile_pool(name="rpool", bufs=2) as rpool,
            tc.tile_pool(name="opool", bufs=3) as opool,
            tc.tile_pool(name="pmain", bufs=2, space="PSUM") as pmain,
            tc.tile_pool(name="paux", bufs=2, space="PSUM") as paux,
            tc.tile_pool(name="dram", bufs=2, space="DRAM") as dram,
            tc.tile_pool(name="dramsh", bufs=2, space="DRAM") as dramsh,
        ):
            ones = const.tile([1, P], F32)
            nc.vector.memset(ones[:], 1.0)
            ident = const.tile([P, P], F32)
            make_identity(nc, ident)
            eoh_sb = const.tile([P, E], F32)
            nc.sync.dma_start(eoh_sb[:], eoh[:])
            rw_sb = const.tile([P, DC, E], F32R)
            nc.sync.dma_start(rw_sb[:], rw.rearrange("(i p) e -> p i e", p=P))

            for c in range(TC):
                # ---- load this chunk's activations: xT[:, c] ----
                x_sb = xpool.tile([P, DC, t_chunk], F32R, name="x_sb")
                nc.sync.dma_start(
                    x_sb[:],
                    xT[:, c * t_chunk:(c + 1) * t_chunk].rearrange(
                        "(i p) t -> p i t", p=P),
                )

                # ---- router for this chunk -> we_sb [P, NT] ----
                we_sb = rpool.tile([P, NT], F32, name="we_sb")
                for j in range(NT):
                    ps_l = paux.tile([P, t_chunk], F32, name="ps_aux")[:, :E]
                    for d in range(DC):
                        nc.tensor.matmul(
                            ps_l[:],
                            x_sb[:, d, j * P:(j + 1) * P],
                            rw_sb[:, d, :],
                            start=(d == 0),
                            stop=(d == DC - 1),
                        )
                    lg = rpool.tile([P, E], F32, name="lg")
                    nc.vector.tensor_copy(lg[:], ps_l[:])
                    m1 = rpool.tile([P, 1], F32, name="m1")
                    nc.vector.reduce_max(m1[:], lg[:], axis=AX.X)
                    mk = rpool.tile([P, E], F32, name="mk")
                    # mk = (lg >= m1) * BIG
                    nc.vector.tensor_scalar(mk[:], lg[:], m1[:], BIG,
                                            op0=ALU.is_ge, op1=ALU.mult)
                    msk = rpool.tile([P, E], F32, name="msk")
                    nc.vector.tensor_sub(msk[:], lg[:], mk[:])
                    m2 = rpool.tile([P, 1], F32, name="m2")
                    nc.vector.reduce_max(m2[:], msk[:], axis=AX.X)
                    # nb = -(m1 + m2)
                    nb = rpool.tile([P, 1], F32, name="nb")
                    nc.vector.tensor_scalar(nb[:], m1[:], m2[:], -1.0,
                                            op0=ALU.add, op1=ALU.mult)
                    # sg = sigmoid(2*lg - m1 - m2)
                    sg = rpool.tile([P, E], F32, name="sg")
                    nc.scalar.activation(sg[:], lg[:], ACTF.Sigmoid,
                                         bias=nb[:], scale=2.0)
                    # keep = (lg >= m2); wsel = sg * keep * eoh
                    keep = rpool.tile([P, E], F32, name="keep")
                    nc.vector.tensor_scalar(keep[:], lg[:], m2[:], None,
                                            op0=ALU.is_ge)
                    wsel = rpool.tile([P, E], F32, name="wsel")
                    nc.vector.tensor_mul(wsel[:], sg[:], keep[:])
                    nc.vector.tensor_mul(wsel[:], wsel[:], eoh_sb[:])
                    nc.vector.reduce_sum(we_sb[:, j:j + 1], wsel[:], axis=AX.X)

                # ---- we broadcast across partitions: [P, t_chunk] ----
                ps_t = paux.tile([P, t_chunk], F32, name="ps_aux")[:NT, :P]
                nc.tensor.transpose(ps_t[:], we_sb[:], ident[:])
                weT = rpool.tile([NT, P], F32, name="weT")
                nc.vector.tensor_copy(weT[:], ps_t[:])
                weflat = rpool.tile([1, NT, P], F32, name="weflat")
                nc.sync.dma_start(weflat[:], weT[:])
                ps_b = paux.tile([P, t_chunk], F32, name="ps_aux")
                for j in range(NT):
                    nc.tensor.matmul(
                        ps_b[:, j * P:(j + 1) * P],
                        ones[:],
                        weflat[:, j, :],
                        start=True,
                        stop=True,
                    )
                we_bc = rpool.tile([P, t_chunk], F32, name="we_bc")
                nc.vector.tensor_copy(we_bc[:], ps_b[:])

                # ---- phase 1: gT[f_tile] = silu(w1.T x) * (v1.T x) ----
                gts = []
                for f in range(FT):
                    w1_cb = wpool.tile([P, DC, P], F32R, name="w1_cb")
                    nc.sync.dma_start(
                        w1_cb[:],
                        w1[:, f * P:(f + 1) * P].rearrange(
                            "(i p) f -> p i f", p=P),
                    )
                    v1_cb = wpool.tile([P, DC, P], F32R, name="v1_cb")
                    nc.sync.dma_start(
                        v1_cb[:],
                        v1[:, f * P:(f + 1) * P].rearrange(
                            "(i p) f -> p i f", p=P),
                    )
                    ps_h = pmain.tile([P, t_chunk], F32, name="ps_h")
                    for d in range(DC):
                        nc.tensor.matmul(ps_h[:], w1_cb[:, d, :], x_sb[:, d, :],
                                         start=(d == 0), stop=(d == DC - 1))
                    ps_v = pmain.tile([P, t_chunk], F32, name="ps_v")
                    for d in range(DC):
                        nc.tensor.matmul(ps_v[:], v1_cb[:, d, :], x_sb[:, d, :],
                                         start=(d == 0), stop=(d == DC - 1))
                    sl = opool.tile([P, t_chunk], F32, name="sl")
                    nc.scalar.activation(sl[:], ps_h[:], ACTF.Silu)
                    gt = gpool.tile([P, t_chunk], F32R, name="gt")
                    nc.vector.tensor_mul(gt[:], sl[:], ps_v[:])
                    gts.append(gt)

                # ---- phase 2: outT[d_tile] = sum_f w2[f,d].T gT[f] ----
                rs_in = dram.tile([D, t_chunk], F32, name="rs_in")
                for dt in range(DT):
                    w2_cb = w2pool.tile([P, FT, P], F32R, name="w2_cb")
                    nc.sync.dma_start(
                        w2_cb[:],
                        w2[:, dt * P:(dt + 1) * P].rearrange(
                            "(i p) d -> p i d", p=P),
                    )
                    ps_o = pmain.tile([P, t_chunk], F32, name="ps_o")
                    for f in range(FT):
                        nc.tensor.matmul(ps_o[:], w2_cb[:, f, :],
                                         gts[f][:],
                                         start=(f == 0), stop=(f == FT - 1))
                    ob = opool.tile([P, t_chunk], F32, name="ob")
                    nc.vector.tensor_mul(ob[:], ps_o[:], we_bc[:])
                    nc.sync.dma_start(rs_in[dt * P:(dt + 1) * P, :], ob[:])

                # ---- reduce-scatter partials over 8 cores ----
                rs_out = dramsh.tile([DS, t_chunk], F32, name="rs_out")
                nc.gpsimd.collective_compute(
                    "ReduceScatter",
                    ALU.add,
                    replica_groups=[list(range(N_CORES))],
                    ins=[rs_in[:].opt()],
                    outs=[rs_out[:].opt()],
                )
                nc.sync.dma_start(out_shards[c], rs_out[:])

    nc.finalize()
    return nc


_CACHE = {}
LAST_RESULTS = None


def _get_nc(T, D, F, E, t_chunk):
    key = (T, D, F, E, t_chunk)
    if key not in _CACHE:
        _CACHE[key] = build(*key)
    return _CACHE[key]


def run_moe(hidden_states, router_w, w1, v1, w2, t_chunk=512):
    global LAST_RESULTS
    _install_trace_hook_if_requested()

    B, S, D = hidden_states.shape
    E = router_w.shape[1]
    F = w1.shape[2]
    T = B * S
    DS = D // N_CORES
    TCN = T // t_chunk

    x = np.ascontiguousarray(hidden_states.reshape(T, D).astype(np.float32))
    xT = np.ascontiguousarray(x.T)
    rwc = np.ascontiguousarray(router_w.astype(np.float32))

    nc = _get_nc(T, D, F, E, t_chunk)

    in_maps = []
    for r in range(N_CORES):
        ohr = np.zeros((P, E), dtype=np.float32)
        ohr[:, r] = 1.0
        in_maps.append({
            "xT": xT,
            "w1": np.ascontiguousarray(w1[r].astype(np.float32)),
            "v1": np.ascontiguousarray(v1[r].astype(np.float32)),
            "w2": np.ascontiguousarray(w2[r].astype(np.float32)),
            "rw": rwc,
            "eoh": ohr,
        })

    res = run_bass_kernel_spmd(nc, in_maps, core_ids=list(range(N_CORES)))
    LAST_RESULTS = res

    fullT = np.empty((D, T), dtype=np.float32)
    for r in range(N_CORES):
        sh = res.results[r]["out_shards"]  # [TCN, DS, t_chunk]
        for c in range(TCN):
            fullT[r * DS:(r + 1) * DS, c * t_chunk:(c + 1) * t_chunk] = sh[c]
    return np.ascontiguousarray(fullT.T).reshape(B, S, D)


def kernel(hidden_states, router_w, w1, v1, w2):
    return run_moe(hidden_states, router_w, w1, v1, w2, t_chunk=512)
